# revision 23
# baseline (speedup 1.0000x reference)
"""Trainium2 Bass kernel for nn_CASCADES_v8_ResonantCore (moe_routing).

Computation (per batch b):
    centroid = 0.7*x[b,-1,:] + 0.3*mean_s(x[b])
    w = softmax(cos_sim(centroid, core_keys)/TEMP)      # [K]
    Lam = sum_k w[k] * core_pool[k]                     # [R,R]
    out[b] = ((x[b] @ V^T) @ Lam^T) @ U^T               # [S,D]

Strategy (8 cores, data-parallel over (batch, seq-half)):
  - K1 (read pass): each core streams its [2048, 4096] shard of x once
    (SWDGE cast-DMA f32->bf16, 1024-col pieces), computes xv^T = V @ x^T
    via PE transpose + band matmuls, and accumulates per-partition column
    sums in a bf16 SBUF tile.  Outputs: xvt bands [128, 2048] bf16 and
    the raw accumulator [128, 4096] bf16 (host finishes the 128-way sum).
  - Host: combines partial colsums, does the tiny routing math (cosine/
    softmax over 16 numbers), folds Lam into W = U @ Lam, sums xvt bands.
  - K2 (write pass): out = xv @ W^T streamed from a single fused [8,6144]
    bf16 input; the output is written in bf16 (host upcasts to f32),
    halving write traffic vs f32.  Correctness budget (rel < 2e-2) has
    ample room for the extra bf16 rounding.
"""

import sys

sys.path.insert(0, "/opt/trn_rl_repo")

import contextlib

import ml_dtypes
import numpy as np

import concourse.bass as bass  # noqa: F401  (registers bass types)
import concourse.tile as tile
from concourse import bacc, mybir
from concourse.bass_utils import run_bass_kernel_spmd

BF16 = ml_dtypes.bfloat16

B, S, D, R, K = 4, 4096, 4096, 8, 4
NCORES = 8
SH = S // 2  # rows of x per core
EPS, TEMP = 1e-8, 0.05

_cache = {}


def build_k1(sh=SH, d=D, r=R):
    """Read pass: xs [sh, d] f32 -> xvt [128, sh//128*128] bf16 bands, acc
    [128, d] bf16 (host reduces partitions for the colsum).

    xvt layout: column block i holds strip i's xv^T in 4 row-bands at
    partitions {0,32,64,96}+0..7 (host sums the bands).
    """
    nstrip = sh // 128          # 16 strips of 128 seq rows
    ngrp = d // 512             # 8 groups of 4 chunks per strip
    nc = bacc.Bacc("TRN2", target_bir_lowering=False, debug=False)
    xs = nc.dram_tensor("xs", [sh, d], mybir.dt.float32, kind="ExternalInput").ap()
    vt = nc.dram_tensor("vt", [128, (d // 128) * r], mybir.dt.bfloat16,
                        kind="ExternalInput").ap()
    idn = nc.dram_tensor("idn", [128, 128], mybir.dt.bfloat16,
                         kind="ExternalInput").ap()
    xvt_out = nc.dram_tensor("xvt", [128, nstrip * 128], mybir.dt.bfloat16,
                             kind="ExternalOutput").ap()
    acc_out = nc.dram_tensor("acc", [128, d], mybir.dt.bfloat16,
                             kind="ExternalOutput").ap()

    with tile.TileContext(nc) as tc:
        with contextlib.ExitStack() as ctx:
            cpool = ctx.enter_context(tc.tile_pool(name="consts", bufs=1))
            xpool2 = ctx.enter_context(tc.tile_pool(name="x2", bufs=10))
            xpool4 = ctx.enter_context(tc.tile_pool(name="x4", bufs=2))
            tpool = ctx.enter_context(tc.tile_pool(name="xT4", bufs=8))
            psT = ctx.enter_context(tc.tile_pool(name="psT", bufs=4, space="PSUM"))
            psX = ctx.enter_context(tc.tile_pool(name="psX", bufs=3, space="PSUM"))

            vt_sb = cpool.tile([128, (d // 128) * r], mybir.dt.bfloat16)
            nc.sync.dma_start(vt_sb[:], vt[:])
            idn_sb = cpool.tile([128, 128], mybir.dt.bfloat16)
            nc.sync.dma_start(idn_sb[:], idn[:])
            xvt_sb = cpool.tile([128, nstrip * 128], mybir.dt.bfloat16)
            nc.vector.memset(xvt_sb[:], 0.0)
            acc = cpool.tile([128, d], mybir.dt.bfloat16)

            def flush_bands(pend):
                # xv^T bands -> staging.  Deferred one strip so these copies
                # (gated on the strip's LAST mm1) never sit ahead of the next
                # strip's colsum adds in the engine queues -- that ordering
                # stalls xq recycling and throttles the read stream.
                if pend is None:
                    return
                pi, p_ps = pend
                for k in range(4):
                    eng = nc.vector.tensor_copy if k % 2 == 0 else nc.scalar.copy
                    eng(
                        xvt_sb[32 * k:32 * k + r, pi * 128:(pi + 1) * 128],
                        p_ps[32 * k:32 * k + r, :],
                    )

            pending = None
            for i in range(nstrip):
                # pieces: 1MB halves mid-stream (best SWDGE rate); quarters
                # on the first/last strip to shorten the ramp/tail
                npiece = 4 if i in (0, nstrip - 1) else 2
                pw = d // npiece
                gpp = pw // 512
                xqs = []
                for p in range(npiece):
                    xpool = xpool4 if npiece == 4 else xpool2
                    xq = xpool.tile([128, pw], mybir.dt.bfloat16,
                                    tag=f"xq{npiece}_{p}")
                    nc.gpsimd.dma_start(
                        xq[:], xs[i * 128:(i + 1) * 128, p * pw:(p + 1) * pw]
                    )
                    xqs.append(xq)
                    # colsum accumulate on vector
                    if i == 0:
                        nc.vector.tensor_copy(acc[:, p * pw:(p + 1) * pw], xq[:])
                    else:
                        nc.vector.tensor_add(
                            acc[:, p * pw:(p + 1) * pw],
                            acc[:, p * pw:(p + 1) * pw], xq[:],
                        )
                if i == nstrip - 1:
                    # gpsimd queue is idle after the last read: runs in
                    # parallel with the sync-queue xvt write below
                    nc.gpsimd.dma_start(acc_out[:], acc[:])
                flush_bands(pending)
                pending = None

                ps_xvt = psX.tile([128, 128], mybir.dt.float32, tag="psxvt")
                xT4s = []

                def mm1(gg):
                    # 4 concurrent col-group matmuls: band k accumulates
                    # chunks c=4g+k over g.  One psum group per band:
                    # start only on the very first write, stop on the last.
                    for k in range(4):
                        c = 4 * gg + k
                        nc.tensor.matmul(
                            ps_xvt[32 * k:32 * k + r, :],
                            vt_sb[:, c * r:(c + 1) * r],
                            xT4s[gg][:, k * 128:(k + 1) * 128],
                            start=(gg == 0),
                            stop=(gg == ngrp - 1),
                            tile_position=(0, 32 * k),
                        )

                for g in range(ngrp):
                    piece, lg = g // gpp, g % gpp
                    xq = xqs[piece]
                    psTt = psT.tile([128, 512], mybir.dt.float32, tag="psT")
                    for k in range(4):
                        cc = lg * 4 + k
                        # transpose chunk: psT[:, k] = xc^T (matmul vs identity)
                        nc.tensor.matmul(
                            psTt[:, k * 128:(k + 1) * 128],
                            xq[:, cc * 128:(cc + 1) * 128], idn_sb[:],
                            start=(k == 0), stop=(k == 3),
                        )
                    xT4 = tpool.tile([128, 512], mybir.dt.bfloat16, tag="xT4")
                    # alternation: vector 1/3, scalar 2/3 mid-stream (vector
                    # also does the colsum adds); 50/50 on the last two strips
                    # where the adds are done and the tail drains
                    if (g % 2 == 0) if i >= nstrip - 2 else (g % 3 == 0):
                        nc.vector.tensor_copy(xT4[:], psTt[:])
                    else:
                        nc.scalar.copy(xT4[:], psTt[:])
                    xT4s.append(xT4)
                    if g >= 1:
                        mm1(g - 1)  # one-group software pipeline skew
                mm1(ngrp - 1)
                pending = (i, ps_xvt)
            flush_bands(pending)
            nc.sync.dma_start(xvt_out[:], xvt_sb[:])

    nc.compile()
    return nc


def build_k2(sh=SH, d=D, r=R):
    """Write pass: out [sh, d] bf16 = xv @ W^T from fused xw [r, 6144] input.

    xw cols [0, 2048): xvt (xv^T), cols [2048, 6144): wt (W^T), bf16.
    The [r=8, 6144] input is replicated on-device to four 32-partition
    row bands; strip i runs in PE row-group i%4, so 4 strips' matmuls
    execute concurrently (the K=8 contraction uses 8 of 128 PE rows --
    row-group tiling recovers ~3-4x matmul throughput).
    Output bf16; host upcasts.
    """
    nsx, ndj = sh // 128, d // 512
    nc = bacc.Bacc("TRN2", target_bir_lowering=False, debug=False)
    xw = nc.dram_tensor("xw", [r, nsx * 128 + d], mybir.dt.bfloat16,
                        kind="ExternalInput").ap()
    out = nc.dram_tensor("out", [sh, d], mybir.dt.bfloat16,
                         kind="ExternalOutput").ap()
    woff = nsx * 128  # 2048

    with tile.TileContext(nc) as tc:
        with contextlib.ExitStack() as ctx:
            cpool = ctx.enter_context(tc.tile_pool(name="consts", bufs=1))
            # 4 tags per pool: 2 bufs x 4 tags = 8 live tiles (psum: 8 banks)
            opool = ctx.enter_context(tc.tile_pool(name="ob", bufs=2))
            psP = ctx.enter_context(tc.tile_pool(name="ps2", bufs=2, space="PSUM"))

            xw_sb = cpool.tile([128, woff + d], mybir.dt.bfloat16)
            for k in range(4):
                # two HWDGE queues so the 4 replica loads overlap
                eng = nc.sync if k % 2 == 0 else nc.scalar
                eng.dma_start(xw_sb[32 * k:32 * k + r, :], xw[:])

            # pair-write view: vout[pair] is a [128, 2, 4096] AP whose
            # partition line p covers out rows {pair*256+p, pair*256+128+p}
            # -- one 2MB DMA per strip pair with 8KB descriptor runs
            vout = out.rearrange("(pair blk p) d -> pair p blk d",
                                 pair=sh // 256, blk=2, p=128)

            for a in range(nsx // 4):          # blocks of 4 strips
                last = a == nsx // 4 - 1
                obs = []
                for pp in range(2):
                    ob = opool.tile([128, 2 * d], mybir.dt.bfloat16,
                                    tag=f"ob{pp}")
                    obs.append(ob)
                for j in range(ndj):           # rounds: 4 concurrent matmuls
                    pss = []
                    for k in range(4):
                        i = 4 * a + k
                        ps2 = psP.tile([128, 512], mybir.dt.float32,
                                       tag=f"ps2_{k}")
                        nc.tensor.matmul(
                            ps2[:],
                            xw_sb[32 * k:32 * k + r, i * 128:(i + 1) * 128],
                            xw_sb[32 * k:32 * k + r,
                                  woff + j * 512:woff + (j + 1) * 512],
                            start=True, stop=True,
                            tile_position=(32 * k, 0),
                        )
                        pss.append(ps2)
                    for k in range(4):
                        # strip 4a+k -> pair tile k//2, col block k%2
                        dst = obs[k // 2][:, (k % 2) * d + j * 512:
                                          (k % 2) * d + (j + 1) * 512]
                        if (j + k) % 2 == 0:
                            nc.vector.tensor_copy(dst, pss[k][:])
                        else:
                            nc.scalar.copy(dst, pss[k][:])
                    if last and (j == ndj // 2 - 1 or j == ndj - 1):
                        # final block: drain half-strips for a short tail
                        hh = 0 if j == ndj // 2 - 1 else 1
                        for k in range(4):
                            i = 4 * a + k
                            nc.sync.dma_start(
                                out[i * 128:(i + 1) * 128,
                                    hh * (d // 2):(hh + 1) * (d // 2)],
                                obs[k // 2][:, (k % 2) * d + hh * (d // 2):
                                            (k % 2) * d + (hh + 1) * (d // 2)],
                            )
                if not last:
                    for pp in range(2):
                        nc.sync.dma_start(vout[2 * a + pp], obs[pp][:])
    nc.compile()
    return nc


def _get_kernels():
    if "k1" not in _cache:
        _cache["k1"] = build_k1()
        _cache["k2"] = build_k2()
    return _cache["k1"], _cache["k2"]


def _vt_layout(V, d, r):
    """[128, (d//128)*r] bf16 with vt[p, c*r + j] = V[j, c*128 + p]."""
    nch = d // 128
    # V [r, d] -> [r, nch, 128] -> [128, nch, r]
    return np.ascontiguousarray(
        V.reshape(r, nch, 128).transpose(2, 1, 0).reshape(128, nch * r)
    ).astype(BF16)


def _idn_layout():
    return np.eye(128, dtype=np.float32).astype(BF16)


def _routing_host(x, core_keys, core_pool, U_shared, cs_by_batch):
    """Per-batch W^T [r, d] bf16 from combined colsums."""
    wt_b = []
    kk = core_keys.astype(np.float64)
    k_n = kk / np.maximum(np.linalg.norm(kk, axis=-1, keepdims=True), EPS)
    for b in range(B):
        mean = cs_by_batch[b] / S
        centroid = 0.7 * x[b, -1, :].astype(np.float64) + 0.3 * mean
        c_n = centroid / max(np.linalg.norm(centroid), EPS)
        sim = c_n @ k_n.T  # [K]
        logits = sim / TEMP
        e = np.exp(logits - logits.max())
        w = e / e.sum()
        Lam = np.einsum("k,kij->ij", w, core_pool.astype(np.float64))  # [R, R]
        W = U_shared.astype(np.float64) @ Lam  # [D, R]
        wt_b.append(np.ascontiguousarray(W.T).astype(BF16))  # [R, D]
    return wt_b


def kernel(x, V_shared, U_shared, core_pool, core_keys):
    x = np.asarray(x)
    V_shared = np.asarray(V_shared)
    U_shared = np.asarray(U_shared)
    core_pool = np.asarray(core_pool)
    core_keys = np.asarray(core_keys)

    nc1, nc2 = _get_kernels()
    core_ids = list(range(NCORES))

    vt_np = _vt_layout(V_shared.astype(np.float32), D, R)
    idn_np = _idn_layout()

    in_maps1 = []
    for c in core_ids:
        b, h = c // 2, c % 2
        xs = np.ascontiguousarray(x[b, h * SH:(h + 1) * SH, :], dtype=np.float32)
        in_maps1.append({"xs": xs, "vt": vt_np, "idn": idn_np})
    res1 = run_bass_kernel_spmd(nc1, in_maps1, core_ids).results

    # --- host routing (tiny: 16 numbers through softmax) ---
    # colsum: acc [128, d] bf16, reduce partitions on host
    cs = [res1[c]["acc"].astype(np.float64).sum(axis=0) for c in core_ids]  # [d]
    # xvt bands: [128, SH] bf16, bands at partitions {0,32,64,96}+0..7
    xvt = [
        res1[c]["xvt"].astype(np.float64).reshape(4, 32, SH)[:, :R].sum(axis=0)
        for c in core_ids
    ]  # [r, SH]

    cs_by_batch = [cs[2 * b] + cs[2 * b + 1] for b in range(B)]
    wt_b = _routing_host(x, core_keys, core_pool, U_shared, cs_by_batch)

    in_maps2 = []
    for c in core_ids:
        b = c // 2
        xw = np.concatenate([xvt[c].astype(BF16), wt_b[b]], axis=1)
        in_maps2.append({"xw": np.ascontiguousarray(xw)})
    res2 = run_bass_kernel_spmd(nc2, in_maps2, core_ids).results

    out = np.empty((B, S, D), dtype=np.float32)
    for c in core_ids:
        b, h = c // 2, c % 2
        out[b, h * SH:(h + 1) * SH, :] = res2[c]["out"].astype(np.float32)
    return out
